# revision 42
# baseline (speedup 1.0000x reference)
"""BinaryTreeCell (binary tree LSTM cell) TRN2 Bass kernel.

Full-input contract: kernel(**inputs) takes the unsharded numpy inputs of
reference.setup_inputs() and returns (c, h), each [131072, 256] float32.

Strategy
--------
Data-parallel over the node dimension N=131072 across 8 NeuronCores
(16384 nodes/core); all weights replicated.

All 14 GEMMs collapse into ONE matmul per node block:
    z   = [x, lh, rh]                 [N, 768]
    A_g = [W_g.T; Ul_g.T; Ur_g.T]     [768, 256]   per gate g in (u,i,lf,rf,o)
    pre = z @ A + b                   [N, 1280]

Matmuls run in fp8 with MatmulPerfMode.DoubleRow (0.5 cycles/row, 2 K-chunks
of 128 per instruction) using a 3-term residual ("Karatsuba") decomposition
that recovers ~bf16 accuracy from fp8 arithmetic:

    z @ A  ~=  z8@A8 + dz8@A8' + z8@dA8
      z8  = e4m3(z)   (subnormals flushed)    A8  = e4m3(A) (flushed)
      dz8 = e5m2(z - z8)                      dA8 = e5m2(A - A8)
                                              A8' = e5m2(A)

(e5m2 for the residuals because the residual magnitudes sit in e4m3's
subnormal range.)  The dz term runs e5m2xe5m2; the dA term runs
mixed-dtype DoubleRow (e4m3 moving x e5m2 stationary), which hardware
accepts and which avoids shipping a third z stream.  Accumulation is
fp32 in PSUM over 9 DoubleRow matmuls per gate-half.  End-to-end rel
error 5.4e-3 (gate 2e-2).

On-chip layout is transposed (features on partitions, nodes on the free
dim).  Gate activations run on ScalarE straight out of PSUM with the
per-partition bias folded in, outputting bf16; c and h are computed on
VectorE in bf16 (2x DVE rate) and DMA'd out as bf16 (host upconverts).

The shared W_fx projection is computed once per half: its PSUM bank is
left with the accumulation group OPEN, snapshotted to SBUF via a ScalarE
Identity activation (for the rf gate's DVE add), then the lf U-part
matmuls continue accumulating into the same bank (42 instead of 45
DoubleRow matmuls per gate-half).  The per-half PE order (fx, rfU, u, i,
o, lfU) hides the cross-engine snapshot latency under u/i/o.

All per-block inputs arrive in ONE u8 DMA (z8|dz8 interleaved) plus
one bf16 DMA for lc/rc, all on the SP queue in exact transfer order;
block 0 runs its matmuls stream-major (full-K) with split z DMAs so
the PE starts after only the first weight/z chunk lands and rides out
the serialized weight preamble.  The final half is full-K in gate order
(i, u, lf, rf, o) so the c-chain and tanh(c) complete during the o
group and only act_o -> h -> DMA trails the last matmul; its c/h
outputs ride HWDGE (lower latency than the Pool SWDGE used elsewhere).
Modeled/measured 300855 ns vs the fp32r baseline's 412022 ns (1.37x).
"""

import numpy as np

N_TOTAL = 131072
D = 256
CORES = 8
NP_ = N_TOTAL // CORES          # 16384 nodes per core
KD = 3 * D                      # 768 contraction
BM = 512                        # node-block (matmul free dim / PSUM bank)
NBLK = NP_ // BM                # 32 blocks per core

_CACHE = {}

_E4_MIN_NORMAL = 2.0 ** -6


def _q_e4m3_flush(t):
    """e4m3 quantization with subnormals flushed to zero, as fp32 values."""
    import ml_dtypes
    tf = t.astype(ml_dtypes.float8_e4m3).astype(np.float32)
    tf[np.abs(tf) < _E4_MIN_NORMAL] = 0.0
    return tf


def _build_nc():
    """Build + compile the per-core Bass program (same NEFF for all cores)."""
    import concourse.tile as tile
    from concourse import bacc, mybir

    f32 = mybir.dt.float32
    bf16 = mybir.dt.bfloat16
    u8 = mybir.dt.uint8
    f8e4 = mybir.dt.float8e4
    f8e5 = mybir.dt.float8e5
    AF = mybir.ActivationFunctionType
    DR = mybir.MatmulPerfMode.DoubleRow

    nc = bacc.Bacc("TRN2", target_bir_lowering=False, debug=False)

    zdz = nc.dram_tensor("zdz", [NBLK, 128, 6, 2, BM], u8,
                         kind="ExternalInput").ap()
    lcrc = nc.dram_tensor("lcrc", [NBLK, 128, 2, 2, BM], bf16,
                          kind="ExternalInput").ap()
    wpk = nc.dram_tensor("wpk", [128, 90, 2, 128], u8,
                         kind="ExternalInput").ap()
    bias = nc.dram_tensor("bias", [128, 10], f32, kind="ExternalInput").ap()
    out = nc.dram_tensor("out", [NBLK, 2, 128, 2, BM], bf16,
                         kind="ExternalOutput").ap()

    stream_dt = [f8e4, f8e5, f8e5]

    with tile.TileContext(nc) as tc:
        with (
            tc.tile_pool(name="wpool", bufs=1) as wpool,
            tc.tile_pool(name="zpool", bufs=4) as zpool,
            tc.tile_pool(name="cpool", bufs=3) as cpool,
            tc.tile_pool(name="gates", bufs=3) as gates,
            tc.tile_pool(name="tmp", bufs=3) as tmp,
            tc.tile_pool(name="outp", bufs=3) as outp,
            tc.tile_pool(name="psum", bufs=8, space="PSUM") as psum,
        ):
            wt = wpool.tile([128, 90, 2, 128], u8, tag="wt")

            # activation-table warmup (hardware loads tables on first use)
            warm = wpool.tile([128, 1], f32, tag="warm")
            nc.gpsimd.memset(warm[:], 0.0)
            warm_o = wpool.tile([128, 1], f32, tag="warm_o")
            nc.scalar.activation(warm_o[:], warm[:], AF.Tanh)
            nc.scalar.activation(warm_o[:], warm[:], AF.Sigmoid)

            b_sb = wpool.tile([128, 10], f32, tag="b")
            nc.gpsimd.dma_start(out=b_sb[:], in_=bias[:])

            def wslot(s, n, kc, dt_s):
                return wt[:, s * 30 + n * 3 + kc, :, :].bitcast(dt_s)

            FULL = [(s, kc) for s in range(3) for kc in range(3)]
            FX = [(s, 0) for s in range(3)]
            UPART = [(s, kc) for s in range(3) for kc in (1, 2)]

            Z_SLOT = [0, 3, 0]          # term 3 reuses the z8e4 slots
            Z_DT = [f8e4, f8e5, f8e4]   # moving dtype per term

            def mm_group(ps, zt, n, parts, m0, bm, start, stop):
                for idx, (s, kc) in enumerate(parts):
                    nc.tensor.matmul(
                        ps[:],
                        wslot(s, n, kc, stream_dt[s]),
                        zt[:, Z_SLOT[s] + kc, :, m0:m0 + bm].bitcast(Z_DT[s]),
                        start=(start and idx == 0),
                        stop=(stop and idx == len(parts) - 1),
                        perf_mode=DR,
                    )

            def half(zt, lt, blk, f, m0=0, bm=BM, last=False):
                n0 = 5 * f
                # fx into B1 (group left open; lfU continues it below)
                ps_lf = psum.tile([128, bm], f32, tag="mm")
                mm_group(ps_lf, zt, n0 + 2, FX, m0, bm, True, False)
                if not last:
                    # snapshot pure fx to SBUF for rf (walrus rejects
                    # psum+=psum)
                    fx_sb = tmp.tile([128, bm], bf16, tag="fx")
                    nc.scalar.activation(fx_sb[:], ps_lf[:], AF.Identity)
                # rfU into B2
                ps_rf = psum.tile([128, bm], f32, tag="mm")
                mm_group(ps_rf, zt, n0 + 3, FULL if last else UPART,
                         m0, bm, True, True)
                # u, i keep the PE busy while the DVE rf-add drains
                ps_u = psum.tile([128, bm], f32, tag="mm")
                mm_group(ps_u, zt, n0 + 0, FULL, m0, bm, True, True)
                rf_t = gates.tile([128, bm], bf16, tag="grf")
                if last:
                    # full-K rf: fewer tail ops after the final matmuls
                    nc.scalar.activation(rf_t[:], ps_rf[:], AF.Sigmoid,
                                         bias=b_sb[:, n0 + 3:n0 + 4])
                else:
                    # rf pre-activation = rfU + fx
                    pre_rf = tmp.tile([128, bm], bf16, tag="prf")
                    nc.vector.tensor_add(pre_rf[:], ps_rf[:], fx_sb[:])
                    nc.scalar.activation(rf_t[:], pre_rf[:], AF.Sigmoid,
                                         bias=b_sb[:, n0 + 3:n0 + 4])
                t3 = tmp.tile([128, bm], bf16, tag="t3")
                nc.vector.tensor_mul(t3[:], rf_t[:], lt[:, 1, f, m0:m0 + bm])
                u_t = gates.tile([128, bm], bf16, tag="gu")
                nc.scalar.activation(u_t[:], ps_u[:], AF.Tanh,
                                     bias=b_sb[:, n0:n0 + 1])
                ps_i = psum.tile([128, bm], f32, tag="mm")
                mm_group(ps_i, zt, n0 + 1, FULL, m0, bm, True, True)
                i_t = gates.tile([128, bm], bf16, tag="gi")
                nc.scalar.activation(i_t[:], ps_i[:], AF.Sigmoid,
                                     bias=b_sb[:, n0 + 1:n0 + 2])
                t1 = tmp.tile([128, bm], bf16, tag="t1")
                nc.vector.tensor_mul(t1[:], i_t[:], u_t[:])
                nc.vector.tensor_add(t1[:], t1[:], t3[:])
                if not last:
                    # mid-stream: o before lfU maximizes PE cover for the
                    # DVE fx-copy WAR hazard on B1
                    ps_o = psum.tile([128, bm], f32, tag="mm")
                    mm_group(ps_o, zt, n0 + 4, FULL, m0, bm, True, True)
                    o_t = gates.tile([128, bm], bf16, tag="go")
                    nc.scalar.activation(o_t[:], ps_o[:], AF.Sigmoid,
                                         bias=b_sb[:, n0 + 4:n0 + 5])
                # lfU accumulates onto fx in B1 (closes the group)
                mm_group(ps_lf, zt, n0 + 2, UPART, m0, bm, False, True)
                lf_t = gates.tile([128, bm], bf16, tag="glf")
                nc.scalar.activation(lf_t[:], ps_lf[:], AF.Sigmoid,
                                     bias=b_sb[:, n0 + 2:n0 + 3])
                t2 = tmp.tile([128, bm], bf16, tag="t2")
                nc.vector.tensor_mul(t2[:], lf_t[:], lt[:, 0, f, m0:m0 + bm])
                ch = outp.tile([128, 2, bm], bf16, tag="ch")
                nc.vector.tensor_add(ch[:, 0, :], t1[:], t2[:])
                tc_t = tmp.tile([128, bm], bf16, tag="tc")
                nc.scalar.activation(tc_t[:], ch[:, 0, :], AF.Tanh)
                if last:
                    ps_o = psum.tile([128, bm], f32, tag="mm")
                    mm_group(ps_o, zt, n0 + 4, FULL, m0, bm, True, True)
                    o_t = gates.tile([128, bm], bf16, tag="go")
                    nc.scalar.activation(o_t[:], ps_o[:], AF.Sigmoid,
                                         bias=b_sb[:, n0 + 4:n0 + 5])
                nc.vector.tensor_mul(ch[:, 1, :], o_t[:], tc_t[:])
                if last:
                    # HWDGE (lower latency than SWDGE) for the kernel tail;
                    # c leaves first so the final transfer is h-only (half
                    # the bytes on the critical chain)
                    nc.sync.dma_start(out=out[blk, f][:, 0:1, m0:m0 + bm],
                                      in_=ch[:, 0:1, :])
                    nc.sync.dma_start(out=out[blk, f][:, 1:2, m0:m0 + bm],
                                      in_=ch[:, 1:2, :])
                else:
                    nc.gpsimd.dma_start(out=out[blk, f][:, :, m0:m0 + bm],
                                        in_=ch[:])

            def last_half(zt, lt, blk, f):
                """All gates full-K, order (i,u,lf,rf,o): the c-chain and
                tanh(c) complete during the o-group, so only act_o -> h ->
                DMA trails the final matmul."""
                n0 = 5 * f
                ps_i = psum.tile([128, BM], f32, tag="mm")
                mm_group(ps_i, zt, n0 + 1, FULL, 0, BM, True, True)
                i_t = gates.tile([128, BM], bf16, tag="gi")
                nc.scalar.activation(i_t[:], ps_i[:], AF.Sigmoid,
                                     bias=b_sb[:, n0 + 1:n0 + 2])
                ps_u = psum.tile([128, BM], f32, tag="mm")
                mm_group(ps_u, zt, n0 + 0, FULL, 0, BM, True, True)
                u_t = gates.tile([128, BM], bf16, tag="gu")
                nc.scalar.activation(u_t[:], ps_u[:], AF.Tanh,
                                     bias=b_sb[:, n0:n0 + 1])
                t1 = tmp.tile([128, BM], bf16, tag="t1")
                nc.vector.tensor_mul(t1[:], i_t[:], u_t[:])
                ps_lf = psum.tile([128, BM], f32, tag="mm")
                mm_group(ps_lf, zt, n0 + 2, FULL, 0, BM, True, True)
                lf_t = gates.tile([128, BM], bf16, tag="glf")
                nc.scalar.activation(lf_t[:], ps_lf[:], AF.Sigmoid,
                                     bias=b_sb[:, n0 + 2:n0 + 3])
                t2 = tmp.tile([128, BM], bf16, tag="t2")
                nc.vector.tensor_mul(t2[:], lf_t[:], lt[:, 0, f, :])
                nc.vector.tensor_add(t1[:], t1[:], t2[:])
                ps_rf = psum.tile([128, BM], f32, tag="mm")
                mm_group(ps_rf, zt, n0 + 3, FULL, 0, BM, True, True)
                rf_t = gates.tile([128, BM], bf16, tag="grf")
                nc.scalar.activation(rf_t[:], ps_rf[:], AF.Sigmoid,
                                     bias=b_sb[:, n0 + 3:n0 + 4])
                t3 = tmp.tile([128, BM], bf16, tag="t3")
                nc.vector.tensor_mul(t3[:], rf_t[:], lt[:, 1, f, :])
                ch = outp.tile([128, 2, BM], bf16, tag="ch")
                nc.vector.tensor_add(ch[:, 0, :], t1[:], t3[:])
                tc_t = tmp.tile([128, BM], bf16, tag="tc")
                nc.scalar.activation(tc_t[:], ch[:, 0, :], AF.Tanh)
                nc.sync.dma_start(out=out[blk, f][:, 0:1, :],
                                  in_=ch[:, 0:1, :])
                ps_o = psum.tile([128, BM], f32, tag="mm")
                mm_group(ps_o, zt, n0 + 4, FULL, 0, BM, True, True)
                o_t = gates.tile([128, BM], bf16, tag="go")
                nc.scalar.activation(o_t[:], ps_o[:], AF.Sigmoid,
                                     bias=b_sb[:, n0 + 4:n0 + 5])
                nc.vector.tensor_mul(ch[:, 1, :], o_t[:], tc_t[:])
                nc.sync.dma_start(out=out[blk, f][:, 1:2, :],
                                  in_=ch[:, 1:2, :])

            def block0_half(zt, lt, blk, f):
                """Stream-major full-K so the PE starts after only the s=0
                weights have arrived."""
                n0 = 5 * f
                ps_a = psum.tile([128, BM], f32, tag="mm")
                ps_b = psum.tile([128, BM], f32, tag="mm")
                ps_c = psum.tile([128, BM], f32, tag="mm")
                ps_d = psum.tile([128, BM], f32, tag="mm")
                ps_e = psum.tile([128, BM], f32, tag="mm")
                pss = [ps_a, ps_b, ps_c, ps_d, ps_e]
                for s in range(3):
                    for g in range(5):
                        mm_group(pss[g], zt, n0 + g,
                                 [(s, kc) for kc in range(3)], 0, BM,
                                 start=(s == 0), stop=(s == 2))
                g_sb = []
                for g in range(5):
                    gt = gates.tile([128, BM], bf16,
                                    tag=("gu", "gi", "glf", "grf", "go")[g])
                    nc.scalar.activation(gt[:], pss[g][:],
                                         AF.Tanh if g == 0 else AF.Sigmoid,
                                         bias=b_sb[:, n0 + g:n0 + g + 1])
                    g_sb.append(gt)
                u_t, i_t, lf_t, rf_t, o_t = g_sb
                t1 = tmp.tile([128, BM], bf16, tag="t1")
                nc.vector.tensor_mul(t1[:], i_t[:], u_t[:])
                t2 = tmp.tile([128, BM], bf16, tag="t2")
                nc.vector.tensor_mul(t2[:], lf_t[:], lt[:, 0, f, :])
                t3 = tmp.tile([128, BM], bf16, tag="t3")
                nc.vector.tensor_mul(t3[:], rf_t[:], lt[:, 1, f, :])
                nc.vector.tensor_add(t1[:], t1[:], t2[:])
                ch = outp.tile([128, 2, BM], bf16, tag="ch")
                nc.vector.tensor_add(ch[:, 0, :], t1[:], t3[:])
                tc_t = tmp.tile([128, BM], bf16, tag="tc")
                nc.scalar.activation(tc_t[:], ch[:, 0, :], AF.Tanh)
                nc.vector.tensor_mul(ch[:, 1, :], o_t[:], tc_t[:])
                nc.gpsimd.dma_start(out=out[blk, f], in_=ch[:])

            # All input DMAs ride one queue in exact transfer order, so block
            # 0's stream-major halves start as soon as each weight stream
            # lands: z0, wA(s0..s2), lcrc0, wB(s0..s2), z1, lcrc1, ...
            for blk in range(NBLK):
                zt = zpool.tile([128, 6, 2, BM], u8, tag="z")
                if blk == 0:
                    # interleave so the PE can start on (wA-s0, z-s0-kc0)
                    nc.sync.dma_start(out=wt[:, 0:15, :, :],
                                      in_=wpk[:, 0:15, :, :])
                    nc.sync.dma_start(out=zt[:, 0:1, :, :],
                                      in_=zdz[blk][:, 0:1, :, :])
                    nc.sync.dma_start(out=zt[:, 1:3, :, :],
                                      in_=zdz[blk][:, 1:3, :, :])
                    nc.sync.dma_start(out=zt[:, 3:6, :, :],
                                      in_=zdz[blk][:, 3:6, :, :])
                    for s in range(1, 3):
                        nc.sync.dma_start(out=wt[:, s * 30:s * 30 + 15, :, :],
                                          in_=wpk[:, s * 30:s * 30 + 15, :, :])
                elif blk == 1:
                    # split so block 1 (stream-major) starts on the s0 chunk
                    nc.sync.dma_start(out=zt[:, 0:3, :, :],
                                      in_=zdz[blk][:, 0:3, :, :])
                    nc.sync.dma_start(out=zt[:, 3:6, :, :],
                                      in_=zdz[blk][:, 3:6, :, :])
                else:
                    nc.sync.dma_start(out=zt[:], in_=zdz[blk])
                lt = cpool.tile([128, 2, 2, BM], bf16, tag="lcrc")
                if blk == 0:
                    # half-1 weight streams outrank lc/rc (PE-critical vs
                    # DVE-slack): wB s0,s1 first, then lcrc0, then wB s2
                    nc.sync.dma_start(out=wt[:, 15:30, :, :],
                                      in_=wpk[:, 15:30, :, :])
                    nc.sync.dma_start(out=wt[:, 45:60, :, :],
                                      in_=wpk[:, 45:60, :, :])
                    nc.sync.dma_start(out=lt[:], in_=lcrc[blk])
                    nc.sync.dma_start(out=wt[:, 75:90, :, :],
                                      in_=wpk[:, 75:90, :, :])
                    for f in range(2):
                        block0_half(zt, lt, blk, f)
                else:
                    nc.sync.dma_start(out=lt[:], in_=lcrc[blk])
                if blk == 0:
                    pass
                elif blk == 1:
                    for f in range(2):
                        half(zt, lt, blk, f)
                elif blk < NBLK - 1:
                    for f in range(2):
                        half(zt, lt, blk, f)
                else:
                    half(zt, lt, blk, 0)
                    last_half(zt, lt, blk, 1)

    nc.compile()
    return nc


def _pack_weights(W_cx, b_cx, W_ox, b_ox, W_fx, b_fx, W_ix, b_ix,
                  U_ilh, U_irh, U_lflh, U_lfrh, U_rflh, U_rfrh,
                  U_ulh, U_urh, U_olh, U_orh):
    """wpk [128, 90, 2, 128] u8: slot s*30 + n*3 + kc holds the [2, 128]
    DoubleRow weight chunk for stream s (0=A8e4, 1=A8e5, 2=dA8e5),
    gate-half n (=5*half+gate, gates ordered u,i,lf,rf,o), K-chunk kc.
    Logical contraction index k = kc*256 + two*128 + p."""
    import ml_dtypes
    f8e5 = ml_dtypes.float8_e5m2

    gate_mats = [
        (W_cx, U_ulh, U_urh, b_cx),    # u
        (W_ix, U_ilh, U_irh, b_ix),    # i
        (W_fx, U_lflh, U_lfrh, b_fx),  # lf
        (W_fx, U_rflh, U_rfrh, b_fx),  # rf
        (W_ox, U_olh, U_orh, b_ox),    # o
    ]
    Q = np.empty((3, 10, KD, 128), dtype=np.uint8)
    bias = np.empty((128, 10), dtype=np.float32)
    for g, (W, Ul, Ur, b) in enumerate(gate_mats):
        Ag = np.concatenate([W.T, Ul.T, Ur.T], axis=0)  # [768, 256]
        for f in range(2):
            n = 5 * f + g
            Ach = Ag[:, f * 128:(f + 1) * 128]
            A8f = _q_e4m3_flush(Ach)
            Q[0, n] = A8f.astype(ml_dtypes.float8_e4m3).view(np.uint8)
            Q[1, n] = Ach.astype(f8e5).view(np.uint8)
            Q[2, n] = (Ach - A8f).astype(f8e5).view(np.uint8)
            bias[:, n] = b[f * 128:(f + 1) * 128]
    # [s, n, kc, two, p, m] -> [p, s, n, kc, two, m] -> [128, 90, 2, 128]
    wpk = Q.reshape(3, 10, 3, 2, 128, 128).transpose(4, 0, 1, 2, 3, 5)
    wpk = np.ascontiguousarray(wpk).reshape(128, 90, 2, 128)
    return wpk, bias


def _pack_z_streams(x, lh, rh):
    """zdz [CORES, NBLK, 128, 6, 2, BM] u8: slot s*3 + kc is the [2, BM]
    DoubleRow moving chunk for stream s (0=z8e4, 1=dz8e5); the dA term
    reuses the z8e4 slots as its moving operand (mixed-dtype DoubleRow).
    k = kc*256 + two*128 + p; node = core*NP_ + blk*BM + m."""
    import ml_dtypes
    f8e4 = ml_dtypes.float8_e4m3
    f8e5 = ml_dtypes.float8_e5m2

    z = np.concatenate([x, lh, rh], axis=1)  # [N, 768]
    z8f = _q_e4m3_flush(z)
    streams = (
        z8f.astype(f8e4).view(np.uint8),
        (z - z8f).astype(f8e5).view(np.uint8),
    )
    zdz = np.empty((CORES, NBLK, 128, 6, 2, BM), dtype=np.uint8)
    for s, S in enumerate(streams):
        # [core, blk, m, kc, two, p] -> [core, blk, p, kc, two, m]
        arr = S.reshape(CORES, NBLK, BM, 3, 2, 128).transpose(0, 1, 5, 3, 4, 2)
        zdz[:, :, :, s * 3:(s + 1) * 3, :, :] = arr
    return zdz


def _pack_lcrc(lc, rc):
    """lcrc [CORES, NBLK, 128, 2, 2, BM] bf16: [.., lr, half, m]."""
    import ml_dtypes
    bf16 = ml_dtypes.bfloat16
    out = np.empty((CORES, NBLK, 128, 2, 2, BM), dtype=bf16)
    for j, t in enumerate((lc, rc)):
        # [core, blk, m, f, p] -> [core, blk, p, f, m]
        arr = t.astype(bf16).reshape(CORES, NBLK, BM, 2, 128)
        out[:, :, :, j, :, :] = arr.transpose(0, 1, 4, 3, 2)
    return out


def kernel(x, lc, lh, rc, rh,
           W_cx, b_cx, W_ox, b_ox, W_fx, b_fx, W_ix, b_ix,
           U_ilh, U_irh, U_lflh, U_lfrh, U_rflh, U_rfrh,
           U_ulh, U_urh, U_olh, U_orh):
    from concourse.bass_utils import run_bass_kernel_spmd

    x = np.asarray(x, dtype=np.float32)
    lc = np.asarray(lc, dtype=np.float32)
    lh = np.asarray(lh, dtype=np.float32)
    rc = np.asarray(rc, dtype=np.float32)
    rh = np.asarray(rh, dtype=np.float32)

    wpk, bias = _pack_weights(
        np.asarray(W_cx, np.float32), np.asarray(b_cx, np.float32),
        np.asarray(W_ox, np.float32), np.asarray(b_ox, np.float32),
        np.asarray(W_fx, np.float32), np.asarray(b_fx, np.float32),
        np.asarray(W_ix, np.float32), np.asarray(b_ix, np.float32),
        np.asarray(U_ilh, np.float32), np.asarray(U_irh, np.float32),
        np.asarray(U_lflh, np.float32), np.asarray(U_lfrh, np.float32),
        np.asarray(U_rflh, np.float32), np.asarray(U_rfrh, np.float32),
        np.asarray(U_ulh, np.float32), np.asarray(U_urh, np.float32),
        np.asarray(U_olh, np.float32), np.asarray(U_orh, np.float32),
    )
    zdz = _pack_z_streams(x, lh, rh)
    lcrc = _pack_lcrc(lc, rc)

    if "nc" not in _CACHE:
        _CACHE["nc"] = _build_nc()
    nc = _CACHE["nc"]

    in_maps = []
    for c in range(CORES):
        in_maps.append({
            "zdz": zdz[c],
            "lcrc": lcrc[c],
            "wpk": wpk,
            "bias": bias,
        })

    import time as _time
    t0 = _time.time()
    res = None
    for attempt, backoff_s in ((0, 15), (1, 45), (2, None)):
        try:
            res = run_bass_kernel_spmd(nc, in_maps, core_ids=list(range(CORES)))
            break
        except Exception:
            # transient device wedge (e.g. NRT_EXEC_UNIT_UNRECOVERABLE):
            # back off and retry; re-raise on the final attempt
            if backoff_s is None:
                raise
            _time.sleep(backoff_s)
    t1 = _time.time()
    _CACHE["last_wall_s"] = t1 - t0
    _CACHE["last_exec_ns"] = res.exec_time_ns

    c_out = np.empty((N_TOTAL, D), dtype=np.float32)
    h_out = np.empty((N_TOTAL, D), dtype=np.float32)
    for ci in range(CORES):
        sl = slice(ci * NP_, (ci + 1) * NP_)
        o = np.asarray(res.results[ci]["out"])  # [NBLK, 2, 128, 2, BM] bf16
        # [blk, f, p, ch, m] -> [blk, m, f, p]
        c_out[sl] = o[:, :, :, 0, :].transpose(0, 3, 1, 2).reshape(NP_, D)
        h_out[sl] = o[:, :, :, 1, :].transpose(0, 3, 1, 2).reshape(NP_, D)
    return c_out, h_out
